# revision 1
# baseline (speedup 1.0000x reference)
"""NCC loss (9x9x9 box normalized cross-correlation) on 8 TRN2 NeuronCores.

Inputs: y_pred, y_true f32 (2,1,128,128,128). Output: scalar f32 loss.

Sharding: D axis (dim 2) split 4-ways per batch -> 8 slabs of 32 D-slices,
each with a 4-slice halo (host zero-pads volume edges).

Per core, on-chip bf16 with f32 PSUM accumulation:
  vols   : I, J, I*I, J*J, I*J                    (DVE / ACT)
  pass 1 : per-d-slice flip matmul vs 9-band B    -> H box,  [W, (43g, 128)]
  pass 2 : per-group flip matmul vs same B        -> W box,  [(l,d), (43g, 128)]
  pass 3 : weight-stationary block-band [128,96]  -> D box, f32 in PSUM
  pointwise cc + partial-sum accumulation         (DVE / ACT / GPSIMD)
Host: sum per-core partials, loss = -sum / N.

Group scheme: 43 groups of 3 h'-slices; groups 0..41 cover h' 0..125,
group 42 covers h' 125..127 (h'=125 duplicated, deduped in pass 3 by
using only loc 1:3 of the block band for the last chunk).
"""

import math

import numpy as np

import concourse.bacc as bacc
import concourse.bass as bass
import concourse.tile as tile
from concourse import mybir
from concourse.bass_utils import run_bass_kernel_spmd

F32 = mybir.dt.float32
BF16 = mybir.dt.bfloat16
ALU = mybir.AluOpType
ACTF = mybir.ActivationFunctionType

B, D, H, W = 2, 128, 128, 128
DL, PAD = 32, 4
DH = DL + 2 * PAD            # 40
NG = 43
C_SCALE = 32.0 / 729.0       # pass-3 band carries 1/32
EPS_P = 1e-5 / 1024.0
N_TOT = float(B * D * H * W)

_CACHE = {}


def _build():
    nc = bacc.Bacc(trn_type="TRN2", target_bir_lowering=False)

    i_dram = nc.dram_tensor("i_slab", [DH, H, W], F32, kind="ExternalInput")
    j_dram = nc.dram_tensor("j_slab", [DH, H, W], F32, kind="ExternalInput")
    out_dram = nc.dram_tensor("partials", [96, 1], F32, kind="ExternalOutput")

    with tile.TileContext(nc) as tc:
        with (
            tc.tile_pool(name="bands", bufs=1) as bands,
            tc.tile_pool(name="t2", bufs=1) as t2p,
            tc.tile_pool(name="accp", bufs=1) as accp,
            tc.tile_pool(name="ps12", bufs=3, space="PSUM") as ps12,
            tc.tile_pool(name="ps3", bufs=5, space="PSUM") as ps3p,
        ):
            # ---------- band matrices ----------
            # bh[p, j] = 1 iff |p - j| <= 4
            bh = bands.tile([128, 128], BF16)
            nc.gpsimd.memset(bh[:, :], 1.0)
            nc.gpsimd.affine_select(bh[:, :], bh[:, :], pattern=[[-1, 128]],
                                    compare_op=ALU.is_ge, fill=0.0,
                                    base=PAD, channel_multiplier=1)
            nc.gpsimd.affine_select(bh[:, :], bh[:, :], pattern=[[1, 128]],
                                    compare_op=ALU.is_ge, fill=0.0,
                                    base=PAD, channel_multiplier=-1)
            # b3[p, (l, j)] = 1/32 iff 0 <= p - 40l - j <= 8, rows 120+ zero
            b3 = bands.tile([128, 3, 32], BF16)
            nc.gpsimd.memset(b3[:, :, :], 1.0 / 32.0)
            nc.gpsimd.affine_select(b3[:, :, :], b3[:, :, :],
                                    pattern=[[-40, 3], [-1, 32]],
                                    compare_op=ALU.is_ge, fill=0.0,
                                    base=0, channel_multiplier=1)
            nc.gpsimd.affine_select(b3[:, :, :], b3[:, :, :],
                                    pattern=[[40, 3], [1, 32]],
                                    compare_op=ALU.is_ge, fill=0.0,
                                    base=8, channel_multiplier=-1)
            nc.gpsimd.affine_select(b3[:, :, :], b3[:, :, :],
                                    pattern=[[0, 3], [0, 32]],
                                    compare_op=ALU.is_ge, fill=0.0,
                                    base=119, channel_multiplier=-1)

            # ---------- inputs, chunked DMA [H, (D, W)] ----------
            inner = tc.tile_pool(name="inputs", bufs=1)
            inputs = inner.__enter__()
            innerv = tc.tile_pool(name="vols", bufs=2)
            volsp = innerv.__enter__()
            innert = tc.tile_pool(name="t1", bufs=2)
            t1p = innert.__enter__()
            i_f32 = inputs.tile([128, DH, W], F32)
            j_f32 = inputs.tile([128, DH, W], F32)
            i_re = i_dram.rearrange("d h w -> h d w")
            j_re = j_dram.rearrange("d h w -> h d w")
            for q in range(4):
                d0, d1 = q * 10, q * 10 + 10
                nc.sync.dma_start(out=i_f32[:, d0:d1, :], in_=i_re[:, d0:d1, :])
                nc.sync.dma_start(out=j_f32[:, d0:d1, :], in_=j_re[:, d0:d1, :])

            # ---------- t2 staging (all five live) ----------
            t2 = [t2p.tile([128, NG, 128], BF16, tag=f"t2_{v}", name=f"t2_{v}")
                  for v in range(5)]

            def make_vol(kind):
                v = volsp.tile([128, DH, W], BF16, tag="vol")
                for q in range(4):
                    s = slice(q * 10, q * 10 + 10)
                    if kind == "I":
                        nc.vector.tensor_copy(v[:, s, :], i_f32[:, s, :])
                    elif kind == "J":
                        nc.vector.tensor_copy(v[:, s, :], j_f32[:, s, :])
                    elif kind == "I2":
                        nc.scalar.square(v[:, s, :], i_f32[:, s, :])
                    elif kind == "J2":
                        nc.scalar.square(v[:, s, :], j_f32[:, s, :])
                    else:
                        nc.vector.tensor_tensor(out=v[:, s, :],
                                                in0=i_f32[:, s, :],
                                                in1=j_f32[:, s, :],
                                                op=ALU.mult)
                return v

            ncopy = 0
            for vi, kind in enumerate(["I", "J", "I2", "J2", "IJ"]):
                vol = make_vol(kind)

                # ----- pass 1: H filter ----- t1 = [W, (h', d)]
                t1 = t1p.tile([128, 128, DH], BF16, tag="t1")
                for db in range(10):
                    ps = ps12.tile([128, 4, 128], F32, tag="ps12")
                    for k in range(4):
                        nc.tensor.matmul(out=ps[:, k, :],
                                         lhsT=vol[:, db * 4 + k, :],
                                         rhs=bh[:, :])
                    dd = slice(db * 4, db * 4 + 4)
                    outA = t1[:, :, dd].rearrange("p h d -> p d h")
                    if ncopy % 2 == 0:
                        nc.scalar.copy(outA, ps[:, :, :])
                    else:
                        nc.vector.tensor_copy(outA, ps[:, :, :])
                    ncopy += 1

                # ----- pass 2: W filter -----
                # group g < 42: h' = 3g..3g+2 -> M = (l, d) = 120 rows
                # group 42: h' = 126, 127 -> M = 80 rows
                for gb in range(11):
                    gs = list(range(gb * 4, min(gb * 4 + 4, NG)))
                    ps = ps12.tile([128, 4, 128], F32, tag="ps12")
                    pmax = 0
                    for k, g in enumerate(gs):
                        h0, hn = (3 * g, 3) if g < 42 else (126, 2)
                        lhs = t1[:, h0:h0 + hn, :].rearrange(
                            "p l d -> p (l d)")
                        nc.tensor.matmul(out=ps[0:hn * DH, k, :],
                                         lhsT=lhs,
                                         rhs=bh[:, :])
                        pmax = max(pmax, hn * DH)
                    n = 2 if gb == 10 else len(gs)  # groups at 120 rows
                    if ncopy % 2 == 0:
                        nc.scalar.copy(t2[vi][0:120, gs[0]:gs[0] + n, :],
                                       ps[0:120, 0:n, :])
                    else:
                        nc.vector.tensor_copy(
                            t2[vi][0:120, gs[0]:gs[0] + n, :],
                            ps[0:120, 0:n, :])
                    ncopy += 1
                    if gb == 10:
                        nc.vector.tensor_copy(t2[vi][0:80, 42, :],
                                              ps[0:80, 2, :])

            innert.__exit__(None, None, None)
            innerv.__exit__(None, None, None)
            inner.__exit__(None, None, None)

            # ---------- pass 3 + chunked pointwise ----------
            cm_ptw = tc.tile_pool(name="ptw", bufs=2)
            ptw = cm_ptw.__enter__()
            acc_big = accp.tile([96, 512], F32)
            nc.vector.memset(acc_big[:, :], 0.0)
            accs = accp.tile([96, 1], F32)
            sqc = math.sqrt(C_SCALE)
            b3f = b3.rearrange("p l j -> p (l j)")

            for ci in range(12):
                if ci < 10:
                    g0, ng, P, F, Kk = ci * 4, 4, 96, 512, 120
                    lhs3 = b3f[0:120, 0:96]
                elif ci == 10:
                    g0, ng, P, F, Kk = 40, 2, 96, 256, 120
                    lhs3 = b3f[0:120, 0:96]
                else:
                    g0, ng, P, F, Kk = 42, 1, 64, 128, 80
                    lhs3 = b3f[0:80, 0:64]

                ps5 = []
                for v in range(5):
                    pt = ps3p.tile([96, 512], F32, tag="ps3")
                    nc.tensor.matmul(
                        out=pt[0:P, 0:F],
                        lhsT=lhs3,
                        rhs=t2[v][0:Kk, g0:g0 + ng, :].rearrange(
                            "p g w -> p (g w)"))
                    ps5.append(pt)
                psI = ps5[0][0:P, 0:F]
                psJ = ps5[1][0:P, 0:F]
                psI2 = ps5[2][0:P, 0:F]
                psJ2 = ps5[3][0:P, 0:F]
                psIJ = ps5[4][0:P, 0:F]

                qI = ptw.tile([96, 512], BF16, tag="qI", name="qI")[0:P, 0:F]
                qJ = ptw.tile([96, 512], BF16, tag="qJ", name="qJ")[0:P, 0:F]
                sJ = ptw.tile([96, 512], BF16, tag="sJ", name="sJ")[0:P, 0:F]
                nc.scalar.activation(qI, psI, ACTF.Square, scale=sqc)
                nc.scalar.activation(qJ, psJ, ACTF.Square, scale=sqc)
                nc.scalar.copy(sJ, psJ)

                m = ptw.tile([96, 512], BF16, tag="m", name="m")[0:P, 0:F]
                nc.vector.scalar_tensor_tensor(out=m, in0=psI, scalar=C_SCALE,
                                               in1=sJ, op0=ALU.mult,
                                               op1=ALU.mult)
                cross = ptw.tile([96, 512], BF16, tag="cross",
                                 name="cross")[0:P, 0:F]
                nc.vector.tensor_tensor(out=cross, in0=psIJ, in1=m,
                                        op=ALU.subtract)
                iv = ptw.tile([96, 512], BF16, tag="iv", name="iv")[0:P, 0:F]
                jv = ptw.tile([96, 512], BF16, tag="jv", name="jv")[0:P, 0:F]
                nc.vector.tensor_tensor(out=iv, in0=psI2, in1=qI,
                                        op=ALU.subtract)
                nc.vector.tensor_tensor(out=jv, in0=psJ2, in1=qJ,
                                        op=ALU.subtract)

                num = ptw.tile([96, 512], BF16, tag="num",
                               name="num")[0:P, 0:F]
                nc.scalar.activation(num, cross, ACTF.Square)

                den = ptw.tile([96, 512], BF16, tag="den",
                               name="den")[0:P, 0:F]
                nc.gpsimd.tensor_tensor(out=den, in0=iv, in1=jv, op=ALU.mult)
                dene = ptw.tile([96, 512], F32, tag="dene",
                                name="dene")[0:P, 0:F]
                nc.gpsimd.tensor_scalar(out=dene, in0=den, scalar1=EPS_P,
                                        scalar2=None, op0=ALU.add)
                rec = ptw.tile([96, 512], F32, tag="rec",
                               name="rec")[0:P, 0:F]
                nc.vector.reciprocal(out=rec, in_=dene)

                cc = ptw.tile([96, 512], BF16, tag="cc", name="cc")[0:P, 0:F]
                nc.gpsimd.tensor_tensor(out=cc, in0=num, in1=rec, op=ALU.mult)
                nc.gpsimd.tensor_tensor(out=acc_big[0:P, 0:F],
                                        in0=acc_big[0:P, 0:F], in1=cc,
                                        op=ALU.add)

            nc.vector.tensor_reduce(out=accs[:, :], in_=acc_big[:, :],
                                    axis=mybir.AxisListType.X, op=ALU.add)
            nc.sync.dma_start(out=out_dram[:, :], in_=accs[:, :])
            cm_ptw.__exit__(None, None, None)

    nc.compile()
    return nc


def kernel(y_pred: np.ndarray, y_true: np.ndarray) -> np.ndarray:
    y_pred = np.ascontiguousarray(np.asarray(y_pred, dtype=np.float32))
    y_true = np.ascontiguousarray(np.asarray(y_true, dtype=np.float32))

    if "nc" not in _CACHE:
        _CACHE["nc"] = _build()
    nc = _CACHE["nc"]

    in_maps = []
    for core in range(8):
        b = core // 4
        d0 = (core % 4) * DL
        islab = np.zeros((DH, H, W), np.float32)
        jslab = np.zeros((DH, H, W), np.float32)
        lo, hi = d0 - PAD, d0 + DL + PAD
        slo, shi = max(lo, 0), min(hi, D)
        islab[slo - lo:shi - lo] = y_true[b, 0, slo:shi]
        jslab[slo - lo:shi - lo] = y_pred[b, 0, slo:shi]
        in_maps.append({"i_slab": islab, "j_slab": jslab})

    res = run_bass_kernel_spmd(nc, in_maps, core_ids=list(range(8)))
    total = 0.0
    for r in res.results:
        total += float(np.asarray(r["partials"], np.float64).sum())
    return np.float32(-total / N_TOT)


if __name__ == "__main__":
    rng = np.random.default_rng(0)
    yp = rng.standard_normal((B, 1, D, H, W), dtype=np.float32)
    yt = rng.standard_normal((B, 1, D, H, W), dtype=np.float32)
    print("loss:", kernel(yp, yt))



# revision 3
# speedup vs baseline: 1.2364x; 1.2364x over previous
"""NCC loss (9x9x9 box normalized cross-correlation) on 8 TRN2 NeuronCores.

Inputs: y_pred, y_true f32 (2,1,128,128,128). Output: scalar f32 loss.

Sharding: D axis (dim 2) split 4-ways per batch -> 8 slabs of 32 D-slices,
each with a 4-slice halo (host zero-pads volume edges).

Per core, fp16 on-chip with f32 PSUM, band taps = 1/9 so every pass emits
window MEANS (scale cancels exactly in cc = cross^2/(Iv*Jv)):
  vols   : I, J, I*I, J*J, I*J              (ACT converts, DVE products)
  pass 1 : per-d-slice flip matmul vs 9-band bh -> H box   [W, (h', d)]
  pass 2 : per-group flip matmul vs bh          -> W box   [(l,d), (g, w)]
  pass 3 : block-band b3 [120,96]               -> D box, f32 PSUM
  ptw    : cc per voxel; per-partition sums via tensor_tensor_reduce
Host: sum per-core partials, loss = -sum / N.

Group scheme: 43 groups of 3 h'-slices; groups 0..41 cover h' 0..125,
group 42 covers h' 126..127 (band rows limited to 80/64).
"""

import numpy as np

import concourse.bacc as bacc
import concourse.tile as tile
from concourse import mybir
from concourse.bass_utils import run_bass_kernel_spmd

F32 = mybir.dt.float32
FP16 = mybir.dt.float16
ALU = mybir.AluOpType
ACTF = mybir.ActivationFunctionType

B, D, H, W = 2, 128, 128, 128
DL, PAD = 32, 4
DH = DL + 2 * PAD            # 40
NG = 43
TAP = 1.0 / 9.0
N_TOT = float(B * D * H * W)

_CACHE = {}


def _build():
    nc = bacc.Bacc(trn_type="TRN2", target_bir_lowering=False)

    i_dram = nc.dram_tensor("i_slab", [DH, H, W], F32, kind="ExternalInput")
    j_dram = nc.dram_tensor("j_slab", [DH, H, W], F32, kind="ExternalInput")
    out_dram = nc.dram_tensor("partials", [128, 1], F32, kind="ExternalOutput")

    with tile.TileContext(nc) as tc:
        with (
            tc.tile_pool(name="bands", bufs=1) as bands,
            tc.tile_pool(name="t2", bufs=1) as t2p,
            tc.tile_pool(name="accp", bufs=1) as accp,
        ):
            # ---------- band matrices (taps 1/9) ----------
            # bh[p, j] = 1/9 iff |p - j| <= 4
            bh = bands.tile([128, 128], FP16)
            nc.gpsimd.memset(bh[:, :], TAP)
            nc.gpsimd.affine_select(bh[:, :], bh[:, :], pattern=[[-1, 128]],
                                    compare_op=ALU.is_ge, fill=0.0,
                                    base=PAD, channel_multiplier=1)
            nc.gpsimd.affine_select(bh[:, :], bh[:, :], pattern=[[1, 128]],
                                    compare_op=ALU.is_ge, fill=0.0,
                                    base=PAD, channel_multiplier=-1)
            # b3[p, (l, j)] = 1/9 iff 0 <= p - 40l - j <= 8, rows 120+ zero
            b3 = bands.tile([128, 3, 32], FP16)
            nc.gpsimd.memset(b3[:, :, :], TAP)
            nc.gpsimd.affine_select(b3[:, :, :], b3[:, :, :],
                                    pattern=[[-40, 3], [-1, 32]],
                                    compare_op=ALU.is_ge, fill=0.0,
                                    base=0, channel_multiplier=1)
            nc.gpsimd.affine_select(b3[:, :, :], b3[:, :, :],
                                    pattern=[[40, 3], [1, 32]],
                                    compare_op=ALU.is_ge, fill=0.0,
                                    base=8, channel_multiplier=-1)
            nc.gpsimd.affine_select(b3[:, :, :], b3[:, :, :],
                                    pattern=[[0, 3], [0, 32]],
                                    compare_op=ALU.is_ge, fill=0.0,
                                    base=119, channel_multiplier=-1)
            b3f = b3.rearrange("p l j -> p (l j)")

            acc_all = accp.tile([128, 12], F32)
            nc.vector.memset(acc_all[:, :], 0.0)

            # ---------- load + stage A: the five fp16 volumes ----------
            cm_vol = tc.tile_pool(name="vols", bufs=1)
            volsp = cm_vol.__enter__()
            cm_in = tc.tile_pool(name="inputs", bufs=1)
            inputs = cm_in.__enter__()

            i_f32 = inputs.tile([128, DH, W], F32)
            j_f32 = inputs.tile([128, DH, W], F32)
            i_re = i_dram.rearrange("d h w -> h d w")
            j_re = j_dram.rearrange("d h w -> h d w")
            for q in range(4):
                s = slice(q * 10, q * 10 + 10)
                nc.sync.dma_start(out=i_f32[:, s, :], in_=i_re[:, s, :])
                nc.sync.dma_start(out=j_f32[:, s, :], in_=j_re[:, s, :])

            vols = [volsp.tile([128, DH, W], FP16, tag=f"vol{v}",
                               name=f"vol{v}") for v in range(5)]
            vI, vJ, vI2, vJ2, vIJ = vols
            for q in range(4):
                s = slice(q * 10, q * 10 + 10)
                nc.scalar.copy(vI[:, s, :], i_f32[:, s, :])
                nc.scalar.copy(vJ[:, s, :], j_f32[:, s, :])
                nc.vector.tensor_tensor(out=vI2[:, s, :], in0=vI[:, s, :],
                                        in1=vI[:, s, :], op=ALU.mult)
                nc.vector.tensor_tensor(out=vJ2[:, s, :], in0=vJ[:, s, :],
                                        in1=vJ[:, s, :], op=ALU.mult)
                nc.vector.tensor_tensor(out=vIJ[:, s, :], in0=vI[:, s, :],
                                        in1=vJ[:, s, :], op=ALU.mult)
            cm_in.__exit__(None, None, None)

            # ---------- passes 1+2, vol-pipelined ----------
            cm_t1 = tc.tile_pool(name="t1", bufs=2)
            t1p = cm_t1.__enter__()
            cm_ps1 = tc.tile_pool(name="ps1", bufs=2, space="PSUM")
            ps1p = cm_ps1.__enter__()
            cm_ps2 = tc.tile_pool(name="ps2", bufs=2, space="PSUM")
            ps2p = cm_ps2.__enter__()

            t2 = [t2p.tile([128, NG, 128], FP16, tag=f"t2_{v}", name=f"t2_{v}")
                  for v in range(5)]

            rr = [0]

            def copy_rr(dst, src):
                # weighted round-robin over ACT/DVE/Pool: ACT,DVE,ACT,Pool,...
                k = rr[0] % 8
                rr[0] += 1
                if k in (0, 2, 5):
                    nc.scalar.copy(dst, src)
                elif k in (1, 4, 6):
                    nc.vector.tensor_copy(dst, src)
                else:
                    nc.gpsimd.tensor_copy(dst, src)

            def pass1(v):
                t1v = t1p.tile([128, 128, DH], FP16, tag="t1", name=f"t1_{v}")
                for db in range(5):
                    ps = ps1p.tile([128, 8, 128], F32, tag="ps1")
                    for k in range(8):
                        nc.tensor.matmul(out=ps[:, k, :],
                                         lhsT=vols[v][:, db * 8 + k, :],
                                         rhs=bh[:, :])
                    dd = slice(db * 8, db * 8 + 8)
                    copy_rr(t1v[:, :, dd].rearrange("p h d -> p d h"),
                            ps[:, :, :])
                return t1v

            def pass2(v, t1v):
                for gb in range(6):
                    gs = list(range(gb * 8, min(gb * 8 + 8, NG)))
                    ps = ps2p.tile([128, 8, 128], F32, tag="ps2")
                    for k, g in enumerate(gs):
                        h0, hn = (3 * g, 3) if g < 42 else (126, 2)
                        lhs = t1v[:, h0:h0 + hn, :].rearrange(
                            "p l d -> p (l d)")
                        nc.tensor.matmul(out=ps[0:hn * DH, k, :],
                                         lhsT=lhs,
                                         rhs=bh[:, :])
                    if gb < 5:
                        copy_rr(t2[v][0:120, gs[0]:gs[0] + 8, :],
                                ps[0:120, :, :])
                    else:
                        copy_rr(t2[v][0:120, 40:42, :], ps[0:120, 0:2, :])
                        copy_rr(t2[v][0:80, 42, :], ps[0:80, 2, :])

            prev = None
            for v in range(5):
                t1v = pass1(v)
                if prev is not None:
                    pass2(*prev)
                prev = (v, t1v)
            pass2(*prev)

            cm_ps2.__exit__(None, None, None)
            cm_ps1.__exit__(None, None, None)
            cm_t1.__exit__(None, None, None)
            cm_vol.__exit__(None, None, None)

            # ---------- pass 3 + pointwise ----------
            cm_ps3 = tc.tile_pool(name="ps3", bufs=7, space="PSUM")
            ps3p = cm_ps3.__enter__()
            cm_ptw = tc.tile_pool(name="ptw", bufs=2)
            ptw = cm_ptw.__enter__()

            for ci in range(12):
                if ci < 10:
                    g0, ng, P, F, Kk = ci * 4, 4, 96, 512, 120
                    lhs3 = b3f[0:120, 0:96]
                elif ci == 10:
                    g0, ng, P, F, Kk = 40, 2, 96, 256, 120
                    lhs3 = b3f[0:120, 0:96]
                else:
                    g0, ng, P, F, Kk = 42, 1, 64, 128, 80
                    lhs3 = b3f[0:80, 0:64]

                ps5 = []
                for v in range(5):
                    pt = ps3p.tile([96, 512], F32, tag="ps3")
                    nc.tensor.matmul(
                        out=pt[0:P, 0:F],
                        lhsT=lhs3,
                        rhs=t2[v][0:Kk, g0:g0 + ng, :].rearrange(
                            "p g w -> p (g w)"))
                    ps5.append(pt)
                sA = ps5[0][0:P, 0:F]
                sB = ps5[1][0:P, 0:F]
                sC = ps5[2][0:P, 0:F]
                sD = ps5[3][0:P, 0:F]
                sE = ps5[4][0:P, 0:F]

                def st(tag, dt=FP16):
                    return ptw.tile([96, 512], dt, tag=tag,
                                    name=tag)[0:P, 0:F]

                qA, bA, qB = st("qA"), st("bA"), st("qB")
                nc.scalar.activation(qA, sA, ACTF.Square)
                nc.scalar.copy(bA, sA)
                nc.scalar.activation(qB, sB, ACTF.Square)

                Pm = st("Pm")
                nc.vector.scalar_tensor_tensor(out=Pm, in0=sB, scalar=1.0,
                                               in1=bA, op0=ALU.bypass,
                                               op1=ALU.mult)
                Iv, Jv, cross = st("Iv"), st("Jv"), st("cross")
                nc.gpsimd.scalar_tensor_tensor(out=Iv, in0=sC, scalar=1.0,
                                               in1=qA, op0=ALU.bypass,
                                               op1=ALU.subtract)
                nc.gpsimd.scalar_tensor_tensor(out=Jv, in0=sD, scalar=1.0,
                                               in1=qB, op0=ALU.bypass,
                                               op1=ALU.subtract)
                nc.gpsimd.scalar_tensor_tensor(out=cross, in0=sE, scalar=1.0,
                                               in1=Pm, op0=ALU.bypass,
                                               op1=ALU.subtract)
                num = st("num")
                nc.scalar.activation(num, cross, ACTF.Square)
                dene = st("dene", F32)
                nc.vector.scalar_tensor_tensor(out=dene, in0=Iv, scalar=1.0,
                                               in1=Jv, op0=ALU.bypass,
                                               op1=ALU.mult)
                rec = st("rec", F32)
                nc.vector.reciprocal_approx_fast(out=rec, in_=dene)
                ccs = st("ccs")
                with nc.allow_low_precision(reason="cc scratch fp16"):
                    nc.vector.tensor_tensor_reduce(
                        out=ccs, in0=num, in1=rec, scale=1.0, scalar=0.0,
                        op0=ALU.mult, op1=ALU.add,
                        accum_out=acc_all[0:P, ci:ci + 1])

            cm_ptw.__exit__(None, None, None)
            cm_ps3.__exit__(None, None, None)

            accs = accp.tile([128, 1], F32)
            nc.vector.tensor_reduce(out=accs[:, :], in_=acc_all[:, :],
                                    axis=mybir.AxisListType.X, op=ALU.add)
            nc.sync.dma_start(out=out_dram[:, :], in_=accs[:, :])

    nc.compile()
    return nc


def kernel(y_pred: np.ndarray, y_true: np.ndarray) -> np.ndarray:
    y_pred = np.ascontiguousarray(np.asarray(y_pred, dtype=np.float32))
    y_true = np.ascontiguousarray(np.asarray(y_true, dtype=np.float32))

    if "nc" not in _CACHE:
        _CACHE["nc"] = _build()
    nc = _CACHE["nc"]

    in_maps = []
    for core in range(8):
        b = core // 4
        d0 = (core % 4) * DL
        islab = np.zeros((DH, H, W), np.float32)
        jslab = np.zeros((DH, H, W), np.float32)
        lo, hi = d0 - PAD, d0 + DL + PAD
        slo, shi = max(lo, 0), min(hi, D)
        islab[slo - lo:shi - lo] = y_true[b, 0, slo:shi]
        jslab[slo - lo:shi - lo] = y_pred[b, 0, slo:shi]
        in_maps.append({"i_slab": islab, "j_slab": jslab})

    res = run_bass_kernel_spmd(nc, in_maps, core_ids=list(range(8)))
    total = 0.0
    for r in res.results:
        total += float(np.asarray(r["partials"], np.float64).sum())
    return np.float32(-total / N_TOT)


if __name__ == "__main__":
    rng = np.random.default_rng(0)
    yp = rng.standard_normal((B, 1, D, H, W), dtype=np.float32)
    yt = rng.standard_normal((B, 1, D, H, W), dtype=np.float32)
    print("loss:", kernel(yp, yt))
